# revision 27
# baseline (speedup 1.0000x reference)
"""Trainium2 Bass kernel for nn_AGNN (gnn_message_passing).

Data-parallel over the meta-batch dim B=8: one episode per NeuronCore,
small weights replicated. The whole G+1-stage pipeline (PointSimilarity
pre + G generations of MHA / D2PAgg / PointSimilarity2-with-topk) runs in
a single NEFF per core; the host only shards inputs and stacks outputs.
"""

import sys
import types

sys.path.insert(0, "/root/.axon_site/_ro/trn_rl_repo")
sys.path.insert(0, "/root/.axon_site")

import numpy as np

import concourse.bass as bass
import concourse.tile as tile_mod
from concourse import mybir
from concourse.vector_clock import ScopedClock

# ---------------------------------------------------------------------------
# Patch 1: walrus in this toolchain rejects >1 semaphore wait per
# instruction ("Too many sync wait commands").  Tile freely attaches
# several waits to one instruction.  Split the excess into standalone
# single-wait NoOps placed right before the instruction on its own engine
# (the sequencer executes them in order, so semantics are unchanged).
# ---------------------------------------------------------------------------
_MAX_WAITS = 1


def _split_excess_waits(nc, ordered_instructions_by_block):
    for bb_name, insts in ordered_instructions_by_block.items():
        out = []
        changed = False
        for ins in insts:
            si = ins.sync_info
            ow = list(si.on_wait) if (si is not None and si.on_wait) else []
            if len(ow) > _MAX_WAITS:
                changed = True
                for w in ow[:-_MAX_WAITS]:
                    nop = mybir.InstNoOp(name=nc.get_next_instruction_name())
                    nop.engine = ins.engine
                    nop.sync_info = mybir.SyncInfo(on_wait=[w], on_update=[])
                    nc.register_instruction(nop)
                    out.append(nop)
                ins.sync_info = mybir.SyncInfo(
                    on_wait=ow[-_MAX_WAITS:], on_update=list(si.on_update or [])
                )
            out.append(ins)
        if changed:
            if isinstance(insts, list):
                insts[:] = out
            else:
                ordered_instructions_by_block[bb_name] = out


class _TCWWrapper:
    def __init__(self, tc, ordered_instructions_by_block, **kw):
        self._inner = _RealTCW(tc, ordered_instructions_by_block, **kw)
        self._nc = tc.nc
        self._ordered = ordered_instructions_by_block

    def assign_waits(self, bb_name):
        self._inner.assign_waits(bb_name)
        _split_excess_waits(self._nc, self._ordered)

    def __getattr__(self, k):
        return getattr(self._inner, k)


_RealTCW = tile_mod.TileClockWait
if not getattr(tile_mod, "_ant_wait_split_patched", False):
    tile_mod.TileClockWait = _TCWWrapper
    tile_mod._ant_wait_split_patched = True


def _patched_drain_and_barrier(self, tick_clock, wait_clock):
    nc = self.nc
    drain_inst = nc.sync.drain()
    wait_clock.add_sem_waits(
        drain_inst.ins, ScopedClock({None: tick_clock.global_clock})
    )
    si = drain_inst.ins.sync_info
    ow = list(si.on_wait) if (si is not None and si.on_wait) else []
    if len(ow) > _MAX_WAITS:
        bb = nc.cur_bb.bb
        assert bb.instructions[-1] is drain_inst.ins
        bb.instructions.pop()
        for w in ow[:-_MAX_WAITS]:
            nop = mybir.InstNoOp(name=nc.get_next_instruction_name())
            nop.engine = drain_inst.ins.engine
            nop.sync_info = mybir.SyncInfo(on_wait=[w], on_update=[])
            nc.register_instruction(nop)
            bb.instructions.append(nop)
        drain_inst.ins.sync_info = mybir.SyncInfo(
            on_wait=ow[-_MAX_WAITS:], on_update=list(si.on_update or [])
        )
        bb.instructions.append(drain_inst.ins)

    nc.all_engine_barrier()
    assert self.sems is not None
    popped = nc._tile_sem_poison_stack.pop()
    assert popped is self._sem_poison
    nc.clear_and_free_semaphores(list(self.sems.allocated().values()))
    nc.all_engine_barrier()


if not getattr(tile_mod.TileContext, "_ant_drain_patched", False):
    tile_mod.TileContext._drain_and_barrier = _patched_drain_and_barrier
    tile_mod.TileContext._ant_drain_patched = True


# ---------------------------------------------------------------------------
# Patch 2: NTFF profile hook shim (the image's antenv lacks axon_hooks);
# only needed when run with trace=True, harmless otherwise.
# ---------------------------------------------------------------------------
def _install_ntff_hook():
    if "antenv.axon_hooks" in sys.modules:
        return
    mod = types.ModuleType("antenv.axon_hooks")
    state = {"hook": None}
    mod.set_axon_ntff_profile_hook = lambda h: state.__setitem__("hook", h)
    mod.get_axon_ntff_profile_hook = lambda: state["hook"]
    sys.modules["antenv.axon_hooks"] = mod
    try:
        import antenv

        antenv.axon_hooks = mod
    except ImportError:
        pass
    try:
        from trn_agent_boot.trn_boot import _ntff_profile_via_ctypes

        mod.set_axon_ntff_profile_hook(
            _ntff_profile_via_ctypes("/opt/axon/libaxon_pjrt.so")
        )
    except Exception:
        pass


_install_ntff_hook()

# ---------------------------------------------------------------------------
# Problem constants (hardcoded per spec)
# ---------------------------------------------------------------------------
B, N, C, G, H, DK = 8, 128, 128, 2, 4, 32
BN_SCALE = float(1.0 / np.sqrt(1.0 + 1e-5))
NCHUNK = 32          # token chunks per psim (512 pair-tokens each)
SLABS = 4            # i-slabs per chunk
F32 = mybir.dt.float32
F32R = mybir.dt.float32r
F16 = mybir.dt.float16
ALU = mybir.AluOpType
ACT = mybir.ActivationFunctionType


def _psim_weight_names(s):
    return [f"w1_{s}", f"w2a_{s}", f"w2b_{s}", f"w3_{s}", f"b3_{s}"]


def _gen_weight_names(g):
    return [f"wq_{g}", f"wk_{g}", f"d1k0_{g}", f"d1k1_{g}", f"d2k0_{g}", f"d2k1_{g}"]


BLOB32 = [
    ("ident", N), ("omeye", N), ("eye_eps", N),
    ("b3_pre", 1), ("b3_g0", 1), ("b3_g1", 1),
    ("wq_0", N), ("wk_0", N), ("d1k0_0", 2 * C), ("d1k1_0", 2 * C),
    ("d2k0_0", C), ("d2k1_0", C),
    ("wq_1", N), ("wk_1", N), ("d1k0_1", 2 * C), ("d1k1_1", 2 * C),
    ("d2k0_1", C), ("d2k1_1", C),
]
BLOB16 = [
    ("w1_pre", 2 * C), ("w2a_pre", C), ("w2b_pre", C), ("w3_pre", 1),
    ("w1_g0", 2 * C), ("w2a_g0", C), ("w2b_g0", C), ("w3_g0", 1),
    ("w1_g1", 2 * C), ("w2a_g1", C), ("w2b_g1", C), ("w3_g1", 1),
]
CORE32 = [("vpT", N), ("vp_rm", C), ("ep0", N)]


def _offsets(blob):
    out, off = {}, 0
    for name, w in blob:
        out[name] = (off, w)
        off += w
    return out, off


OFF32, TOT32 = _offsets(BLOB32)
OFF16, TOT16 = _offsets(BLOB16)
OFFC, TOTC = _offsets(CORE32)


class _Pools:
    pass


_BLK = 16


def _tri_chunks():
    """Block-triangular chunk table: (i0, k, j0, w) — rows i0..i0+k,
    cols j0..j0+w, exploiting z's exact symmetry (z[i,j] == z[j,i])."""
    chunks = []
    for b in range(N // _BLK):
        j0 = _BLK * b
        w = N - j0
        kmax = max(1, min(_BLK, 512 // w))
        kmin = max(1, -(-256 // w))  # ceil(256/w)
        rem = _BLK
        i0 = j0
        while rem:
            k = min(kmax, rem)
            if 0 < rem - k < kmin:
                k = rem - kmin
            chunks.append((i0, k, j0, w))
            i0 += k
            rem -= k
    return chunks


_TRI = _tri_chunks()


def _emit_psim(nc, P, sb, cfg, vpT, ep_prev, nb, stage):
    """ep_new = PointSimilarity(vpT, ep_prev); nb = #smallest entries to zap."""
    from concourse.bass_types import AP as _AP

    zsb = sb.tile([N, N], F32, tag="zsb")
    zline = sb.tile([1, N * N], F32, tag="zline")
    # ep_nd/ep_sum depend only on ep_prev: compute early, overlaps the chunks
    ep_nd = sb.tile([N, N], F32, tag="ep_nd")
    ep_sum = sb.tile([N, 1], F32, tag="ep_sum")
    nc.vector.scalar_tensor_tensor(
        out=ep_nd[:], in0=ep_prev[:], scalar=1.0, in1=P.omeye[:],
        op0=ALU.mult, op1=ALU.mult, accum_out=ep_sum[:],
    )
    off = 0
    for (i0, k, j0, w) in _TRI:
        nt = k * w
        d = P.work.tile([C, 512], F32, tag="d")
        a = vpT[:]
        rep = _AP(a.tensor, a.offset + j0, [a.ap[0], [0, k], [1, w]])
        bcast = vpT[:, i0:i0 + k].to_broadcast([C, k, w])
        nc.vector.tensor_tensor(
            out=d[:, 0:nt].rearrange("p (s n) -> p s n", s=k),
            in0=rep, in1=bcast, op=ALU.subtract,
        )
        sim = P.work.tile([C, 512], F16, tag="sim")
        nc.gpsimd.tensor_tensor(out=sim[:, 0:nt], in0=d[:, 0:nt], in1=d[:, 0:nt],
                                op=ALU.mult)
        p1 = P.p1.tile([128, 1024], F32, tag="p1")
        nc.tensor.matmul(p1[:, 0:nt], cfg["w1r"][:, 0:128], sim[:, 0:nt],
                         start=True, stop=True)
        nc.tensor.matmul(p1[:, 512:512 + nt], cfg["w1r"][:, 128:256], sim[:, 0:nt],
                         start=True, stop=True)
        h1 = P.work.tile([128, 1024], F16, tag="h1")
        nc.scalar.activation(
            h1[:, 0:2 * nt].rearrange("p (a n) -> p a n", a=2),
            p1[:].rearrange("p (a n) -> p a n", a=2)[:, :, 0:nt],
            ACT.Lrelu, alpha=0.01,
        )
        p2 = P.p2.tile([128, 512], F32, tag="p2")
        nc.tensor.matmul(p2[:, 0:nt], cfg["w2ar"][:], h1[:, 0:nt],
                         start=True, stop=False)
        nc.tensor.matmul(p2[:, 0:nt], cfg["w2br"][:], h1[:, nt:2 * nt],
                         start=False, stop=True)
        h2 = P.work.tile([128, 512], F16, tag="h2")
        nc.scalar.activation(h2[:, 0:nt], p2[:, 0:nt], ACT.Lrelu, alpha=0.01)
        pz = P.pz.tile([1, 512], F32, tag="pz")
        nc.tensor.matmul(pz[:, 0:nt], cfg["w3r"][:], h2[:, 0:nt],
                         start=True, stop=True)
        nc.vector.tensor_copy(zline[:, off:off + nt], pz[:, 0:nt])
        # strip -> zsb block rows [i0:i0+k, j0:j0+w]
        nc.sync.dma_start(zsb[i0:i0 + k, j0:j0 + w], zline[:, off:off + nt])
        off += nt

    # mirror the lower triangle: zsb[i,j] = zsb[j,i] where j < 16*(i//16)
    pT = P.aux.tile([128, 512], F32, tag="pz")
    nc.tensor.transpose(pT[:, 0:128], zsb[:], P.ident[:])
    nc.vector.copy_predicated(zsb[:], P.masklow[:], pT[:, 0:128])

    e = sb.tile([N, N], F32, tag="esig")
    nc.scalar.activation(e[:], zsb[:], ACT.Sigmoid, bias=cfg["b3"][:], scale=1.0)
    s1 = sb.tile([N, 1], F32, tag="l1_s")
    if nb:
        nc.vector.tensor_tensor(out=e[:], in0=e[:], in1=ep_nd[:], op=ALU.mult)
    else:
        nc.vector.scalar_tensor_tensor(out=e[:], in0=e[:], scalar=1.0,
                                       in1=ep_nd[:], op0=ALU.bypass, op1=ALU.mult,
                                       accum_out=s1[:])

    if nb:
        # zap the nb smallest entries per row: run max8/match_replace on 2-e
        x = sb.tile([N, N], F32, tag="topk_x")
        nc.vector.tensor_scalar(out=x[:], in0=e[:], scalar1=-1.0, scalar2=2.0,
                                op0=ALU.mult, op1=ALU.add)
        mx = sb.tile([N, 8], F32, tag="topk_mx")
        done = 0
        while done < nb:
            k = min(8, nb - done)
            nc.vector.max(out=mx[:], in_=x[:])
            if k < 8:
                nc.vector.memset(mx[:, k:], 0.0)
            nc.vector.match_replace(out=x[:], in_to_replace=mx[:], in_values=x[:],
                                    imm_value=0.0)
            done += k
        m = sb.tile([N, N], F32, tag="topk_m")
        nc.vector.tensor_scalar(out=m[:], in0=x[:], scalar1=0.5, scalar2=None,
                                op0=ALU.is_gt)
        nc.vector.scalar_tensor_tensor(out=e[:], in0=e[:], scalar=1.0, in1=m[:],
                                       op0=ALU.bypass, op1=ALU.mult,
                                       accum_out=s1[:])

    rinv = sb.tile([N, 1], F32, tag="l1_rinv")
    nc.vector.reciprocal(rinv[:], s1[:])
    scl = sb.tile([N, 1], F32, tag="l1_scl")
    nc.vector.tensor_tensor(out=scl[:], in0=rinv[:], in1=ep_sum[:], op=ALU.mult)
    e2 = sb.tile([N, N], F32, tag="e2")
    rs = sb.tile([N, 1], F32, tag="rs")
    nc.vector.scalar_tensor_tensor(
        out=e2[:], in0=e[:], scalar=scl[:], in1=P.eye_eps[:],
        op0=ALU.mult, op1=ALU.add, accum_out=rs[:],
    )
    r2 = sb.tile([N, 1], F32, tag="r2")
    nc.vector.reciprocal(r2[:], rs[:])
    ep_new = sb.tile([N, N], F32, tag=f"ep_{stage}")
    nc.vector.tensor_scalar(out=ep_new[:], in0=e2[:], scalar1=r2[:], scalar2=None,
                            op0=ALU.mult)
    return ep_new


def _emit_mha(nc, P, sb, g, vpT, wq, wk):
    pqk = P.aux.tile([128, 512], F32, tag="pz")
    nc.tensor.matmul(pqk[:, 0:128], wq[:], vpT[:], start=True, stop=True)
    nc.tensor.matmul(pqk[:, 128:256], wk[:], vpT[:], start=True, stop=True)
    qkT = sb.tile([C, 2 * N], F32, tag="qkT")
    nc.vector.tensor_copy(qkT[:], pqk[:, 0:256])
    # per-head strips to partitions 0:32 (DMA partition reshape)
    qkh = sb.tile([DK, H, 2 * N], F32, tag="qkh")
    for h in range(H):
        nc.sync.dma_start(qkh[0:DK, h, :], qkT[DK * h:DK * (h + 1), :])
    attn = sb.tile([N, N], F32, tag=f"attn_{g}")
    inv_sqrt_dk = float(1.0 / np.sqrt(DK))
    # all 4 head scores into ONE psum bank, then the 4 Exps back-to-back
    # (minimizes ACT table swaps against the surrounding Lrelu stream)
    ps = P.aux.tile([128, 512], F32, tag="pz")
    for h in range(H):
        nc.tensor.matmul(ps[:, 128 * h:128 * (h + 1)],
                         qkh[0:DK, h, 0:N], qkh[0:DK, h, N:2 * N],
                         start=True, stop=True)
    mxs = sb.tile([N, H], F32, tag="mha_mx")
    nbias = sb.tile([N, H], F32, tag="mha_nb")
    nc.vector.tensor_reduce(
        mxs[:], ps[:].rearrange("p (h n) -> p h n", h=H),
        axis=mybir.AxisListType.X, op=ALU.max)
    nc.vector.tensor_scalar(out=nbias[:], in0=mxs[:], scalar1=-inv_sqrt_dk,
                            scalar2=None, op0=ALU.mult)
    ehs, ses = [], []
    for h in range(H):
        eh = sb.tile([N, N], F32, tag=f"mha_eh{h}")
        se = sb.tile([N, 1], F32, tag=f"mha_se{h}")
        nc.scalar.activation(eh[:], ps[:, 128 * h:128 * (h + 1)], ACT.Exp,
                             bias=nbias[:, h:h + 1], scale=inv_sqrt_dk,
                             accum_out=se[:])
        ehs.append(eh)
        ses.append(se)
    for h in range(H):
        rc = sb.tile([N, 1], F32, tag="mha_rc")
        nc.vector.reciprocal(rc[:], ses[h][:])
        nc.vector.tensor_scalar(out=rc[:], in0=rc[:], scalar1=1.0 / H,
                                scalar2=None, op0=ALU.mult)
        if h == 0:
            nc.vector.tensor_scalar(out=attn[:], in0=ehs[h][:], scalar1=rc[:],
                                    scalar2=None, op0=ALU.mult)
        else:
            nc.vector.scalar_tensor_tensor(out=attn[:], in0=ehs[h][:], scalar=rc[:],
                                           in1=attn[:], op0=ALU.mult, op1=ALU.add)
    # pre-kill the diagonal (d2p's edge needs (ep*attn)*(1-eye))
    nc.vector.tensor_tensor(out=attn[:], in0=attn[:], in1=P.omeye[:], op=ALU.mult)
    return attn


def _emit_d2p(nc, P, sb, g, ep, attn, vpT, vp_rm, cfg, last):
    edge = sb.tile([N, N], F32, tag="edge")
    s = sb.tile([N, 1], F32, tag="d2p_s")
    nc.vector.scalar_tensor_tensor(out=edge[:], in0=ep[:], scalar=1.0, in1=attn[:],
                                   op0=ALU.bypass, op1=ALU.mult, accum_out=s[:])
    rr = sb.tile([N, 1], F32, tag="d2p_rr")
    nc.vector.reciprocal(rr[:], s[:])
    nc.vector.tensor_scalar(out=edge[:], in0=edge[:], scalar1=rr[:], scalar2=None,
                            op0=ALU.mult)
    pT = P.aux.tile([128, 512], F32, tag="pz")
    nc.tensor.transpose(pT[:, 0:128], edge[:], P.ident[:])
    edgeT = sb.tile([N, N], F32, tag="edgeT")
    nc.vector.tensor_copy(edgeT[:], pT[:, 0:128])
    pa = P.aux.tile([128, 512], F32, tag="pz")
    nc.tensor.matmul(pa[:, 0:128], vp_rm[:], edgeT[:], start=True, stop=True)
    aggrT = sb.tile([C, N], F32, tag="aggrT")
    nc.vector.tensor_copy(aggrT[:], pa[:, 0:128])
    hs = []
    for oh in range(2):
        pm = P.aux.tile([128, 512], F32, tag="pz")
        nc.tensor.matmul(pm[:, 0:128], cfg[f"d1k0"][:, C * oh:C * (oh + 1)], vpT[:],
                         start=True, stop=False)
        nc.tensor.matmul(pm[:, 0:128], cfg[f"d1k1"][:, C * oh:C * (oh + 1)], aggrT[:],
                         start=False, stop=True)
        h_oh = sb.tile([C, N], F32, tag=f"d2ph{oh}")
        nc.scalar.activation(h_oh[:], pm[:, 0:128], ACT.Lrelu, alpha=0.01)
        hs.append(h_oh)
    pv = P.aux.tile([128, 512], F32, tag="pz")
    nc.tensor.matmul(pv[:, 0:128], cfg["d2k0"][:], hs[0][:], start=True, stop=False)
    nc.tensor.matmul(pv[:, 0:128], cfg["d2k1"][:], hs[1][:], start=False, stop=True)
    vpT_new = sb.tile([C, N], F32, tag=f"vpT_{g + 1}")
    nc.scalar.activation(vpT_new[:], pv[:, 0:128], ACT.Lrelu, alpha=0.01)
    vp_rm_new = None
    if not last:
        pt2 = P.aux.tile([128, 512], F32, tag="pz")
        nc.tensor.transpose(pt2[:, 0:128], vpT_new[:], P.ident[:])
        vp_rm_new = sb.tile([N, C], F32, tag=f"vprm_{g + 1}")
        nc.vector.tensor_copy(vp_rm_new[:], pt2[:, 0:128])
    return vpT_new, vp_rm_new


def build():
    """Build the single-core Bass graph (SPMD across 8 cores)."""
    nc = bass.Bass()
    b32_ext = nc.declare_dram_parameter("blob32", [128, TOT32], F32, isOutput=False)
    b16_ext = nc.declare_dram_parameter("blob16", [128, TOT16], F16, isOutput=False)
    core_ext = nc.declare_dram_parameter("core32", [128, TOTC], F32, isOutput=False)
    mask_ext = nc.declare_dram_parameter("masklow", [N, N], mybir.dt.uint8,
                                         isOutput=False)
    out_ext = nc.declare_dram_parameter("out", [N, N], F32, isOutput=True)

    with tile_mod.TileContext(nc) as tc:
        with tc.tile_pool(name="const", bufs=1) as const, \
             tc.tile_pool(name="state", bufs=1) as state, \
             tc.tile_pool(name="work", bufs=2) as work, \
             tc.tile_pool(name="p1", bufs=2, space="PSUM") as p1, \
             tc.tile_pool(name="p2", bufs=2, space="PSUM") as p2, \
             tc.tile_pool(name="pz", bufs=2, space="PSUM") as pz:

            P = _Pools()
            P.work, P.p1, P.p2, P.pz, P.aux = work, p1, p2, pz, pz

            # --- four bulk loads (small, psim-critical ones first) ---
            core = const.tile([128, TOTC], F32, tag="core")
            nc.sync.dma_start(core[:], core_ext[:])
            b16 = const.tile([128, TOT16], F16, tag="b16")
            nc.sync.dma_start(b16[:], b16_ext[:])
            P.masklow = const.tile([N, N], mybir.dt.uint8, tag="masklow")
            nc.sync.dma_start(P.masklow[:], mask_ext[:])
            b32 = const.tile([128, TOT32], F32, tag="b32")
            nc.sync.dma_start(b32[:], b32_ext[:])

            def s32(name):
                off, w = OFF32[name]
                return b32[:, off:off + w]

            def s16(name):
                off, w = OFF16[name]
                return b16[:, off:off + w]

            def sc(name):
                off, w = OFFC[name]
                return core[:, off:off + w]

            P.ident = s32("ident")
            P.omeye = s32("omeye")
            P.eye_eps = s32("eye_eps")

            vpT0, vprm0, ep0 = sc("vpT"), sc("vp_rm"), sc("ep0")

            psim_cfgs = {
                s: {"w1r": s16(f"w1_{s}"), "w2ar": s16(f"w2a_{s}"),
                    "w2br": s16(f"w2b_{s}"), "w3r": s16(f"w3_{s}"),
                    "b3": s32(f"b3_{s}")}
                for s in ("pre", "g0", "g1")
            }
            gen_cfgs = {
                g: {k: s32(f"{k}_{g}")
                    for k in ("wq", "wk", "d1k0", "d1k1", "d2k0", "d2k1")}
                for g in range(G)
            }

            # --- the pipeline (mha_g hoisted before the psim it overlaps) ---
            attn = _emit_mha(nc, P, state, 0, vpT0, gen_cfgs[0]["wq"],
                             gen_cfgs[0]["wk"])
            ep = _emit_psim(nc, P, state, psim_cfgs["pre"], vpT0, ep0, 0, "pre")
            vpT, vp_rm = vpT0, vprm0
            for g in range(G):
                vpT, vp_rm = _emit_d2p(nc, P, state, g, ep, attn, vpT, vp_rm,
                                       gen_cfgs[g], last=(g == G - 1))
                if g + 1 < G:
                    attn = _emit_mha(nc, P, state, g + 1, vpT,
                                     gen_cfgs[g + 1]["wq"], gen_cfgs[g + 1]["wk"])
                kval = int(N * (1.0 - 0.1 * (g + 1)))
                ep = _emit_psim(nc, P, state, psim_cfgs[f"g{g}"], vpT, ep,
                                N - kval, f"g{g}")

            nc.sync.dma_start(out_ext[:], ep[:])

    return nc


def make_in_maps(vp, ep0, ps_pre_w1, ps_pre_w2, ps_pre_w3, ps_pre_b3,
                 ps_w1, ps_w2, ps_w3, ps_b3, d2p_w1, d2p_w2, mha_wq, mha_wk):
    f = np.float32
    a = lambda x: np.ascontiguousarray(np.asarray(x), dtype=f)
    eye = np.eye(N, dtype=f)
    ii = np.arange(N)
    masklow = np.ascontiguousarray(
        (ii[None, :] < (ii[:, None] // 16) * 16).astype(np.uint8))

    vals32 = {
        "ident": eye, "omeye": a(1.0 - eye), "eye_eps": a(eye + 1e-6),
    }
    vals16 = {}
    for s, w1, w2, w3, b3 in [
        ("pre", ps_pre_w1, ps_pre_w2, ps_pre_w3, ps_pre_b3),
        ("g0", ps_w1[0], ps_w2[0], ps_w3[0], ps_b3[0]),
        ("g1", ps_w1[1], ps_w2[1], ps_w3[1], ps_b3[1]),
    ]:
        w2s = a(np.asarray(w2) * BN_SCALE)
        vals16[f"w1_{s}"] = a(np.asarray(w1) * BN_SCALE)
        vals16[f"w2a_{s}"] = w2s[:C]
        vals16[f"w2b_{s}"] = w2s[C:]
        vals16[f"w3_{s}"] = a(w3)
        vals32[f"b3_{s}"] = a(np.broadcast_to(np.asarray(b3).reshape(1, 1), (N, 1)))
    for g in range(G):
        d1s = a(np.asarray(d2p_w1[g]) * BN_SCALE)
        d2s = a(np.asarray(d2p_w2[g]) * BN_SCALE)
        vals32[f"wq_{g}"] = a(mha_wq[g])
        vals32[f"wk_{g}"] = a(mha_wk[g])
        vals32[f"d1k0_{g}"] = d1s[:C]
        vals32[f"d1k1_{g}"] = d1s[C:]
        vals32[f"d2k0_{g}"] = d2s[:C]
        vals32[f"d2k1_{g}"] = d2s[C:]

    blob32 = np.zeros((128, TOT32), dtype=f)
    for name, w in BLOB32:
        off = OFF32[name][0]
        blob32[:, off:off + w] = vals32[name]
    blob16 = np.zeros((128, TOT16), dtype=np.float16)
    for name, w in BLOB16:
        off = OFF16[name][0]
        blob16[:, off:off + w] = vals16[name].astype(np.float16)

    shared = {"blob32": blob32, "blob16": blob16, "masklow": masklow}
    vp = a(vp)
    ep0 = a(ep0)
    in_maps = []
    for i in range(B):
        cb = np.zeros((128, TOTC), dtype=f)
        cb[:, OFFC["vpT"][0]:OFFC["vpT"][0] + N] = vp[i].T
        cb[:, OFFC["vp_rm"][0]:OFFC["vp_rm"][0] + C] = vp[i]
        cb[:, OFFC["ep0"][0]:OFFC["ep0"][0] + N] = ep0[i]
        m = dict(shared)
        m["core32"] = cb
        in_maps.append(m)
    return in_maps


_CACHED_NC = None


def _get_nc():
    global _CACHED_NC
    if _CACHED_NC is None:
        _CACHED_NC = build()
    return _CACHED_NC


def run(in_maps, trace=False):
    from concourse.bass_utils import run_bass_kernel_spmd

    nc = _get_nc()
    return run_bass_kernel_spmd(nc, in_maps, list(range(B)), trace=trace)


def kernel(**inputs) -> np.ndarray:
    in_maps = make_in_maps(**inputs)
    r = run(in_maps, trace=False)
    return np.stack([r.results[i]["out"] for i in range(B)]).astype(np.float32)


# revision 28
# speedup vs baseline: 1.2381x; 1.2381x over previous
"""Trainium2 Bass kernel for nn_AGNN (gnn_message_passing).

Data-parallel over the meta-batch dim B=8: one episode per NeuronCore,
small weights replicated. The whole G+1-stage pipeline (PointSimilarity
pre + G generations of MHA / D2PAgg / PointSimilarity2-with-topk) runs in
a single NEFF per core; the host only shards inputs and stacks outputs.
"""

import sys
import types

sys.path.insert(0, "/root/.axon_site/_ro/trn_rl_repo")
sys.path.insert(0, "/root/.axon_site")

import numpy as np

import concourse.bass as bass
import concourse.tile as tile_mod
from concourse import mybir
from concourse.vector_clock import ScopedClock

# ---------------------------------------------------------------------------
# Patch 1: walrus in this toolchain rejects >1 semaphore wait per
# instruction ("Too many sync wait commands").  Tile freely attaches
# several waits to one instruction.  Split the excess into standalone
# single-wait NoOps placed right before the instruction on its own engine
# (the sequencer executes them in order, so semantics are unchanged).
# ---------------------------------------------------------------------------
_MAX_WAITS = 1


def _split_excess_waits(nc, ordered_instructions_by_block):
    for bb_name, insts in ordered_instructions_by_block.items():
        out = []
        changed = False
        for ins in insts:
            si = ins.sync_info
            ow = list(si.on_wait) if (si is not None and si.on_wait) else []
            if len(ow) > _MAX_WAITS:
                changed = True
                for w in ow[:-_MAX_WAITS]:
                    nop = mybir.InstNoOp(name=nc.get_next_instruction_name())
                    nop.engine = ins.engine
                    nop.sync_info = mybir.SyncInfo(on_wait=[w], on_update=[])
                    nc.register_instruction(nop)
                    out.append(nop)
                ins.sync_info = mybir.SyncInfo(
                    on_wait=ow[-_MAX_WAITS:], on_update=list(si.on_update or [])
                )
            out.append(ins)
        if changed:
            if isinstance(insts, list):
                insts[:] = out
            else:
                ordered_instructions_by_block[bb_name] = out


class _TCWWrapper:
    def __init__(self, tc, ordered_instructions_by_block, **kw):
        self._inner = _RealTCW(tc, ordered_instructions_by_block, **kw)
        self._nc = tc.nc
        self._ordered = ordered_instructions_by_block

    def assign_waits(self, bb_name):
        self._inner.assign_waits(bb_name)
        _split_excess_waits(self._nc, self._ordered)

    def __getattr__(self, k):
        return getattr(self._inner, k)


_RealTCW = tile_mod.TileClockWait
if not getattr(tile_mod, "_ant_wait_split_patched", False):
    tile_mod.TileClockWait = _TCWWrapper
    tile_mod._ant_wait_split_patched = True


def _patched_drain_and_barrier(self, tick_clock, wait_clock):
    nc = self.nc
    drain_inst = nc.sync.drain()
    wait_clock.add_sem_waits(
        drain_inst.ins, ScopedClock({None: tick_clock.global_clock})
    )
    si = drain_inst.ins.sync_info
    ow = list(si.on_wait) if (si is not None and si.on_wait) else []
    if len(ow) > _MAX_WAITS:
        bb = nc.cur_bb.bb
        assert bb.instructions[-1] is drain_inst.ins
        bb.instructions.pop()
        for w in ow[:-_MAX_WAITS]:
            nop = mybir.InstNoOp(name=nc.get_next_instruction_name())
            nop.engine = drain_inst.ins.engine
            nop.sync_info = mybir.SyncInfo(on_wait=[w], on_update=[])
            nc.register_instruction(nop)
            bb.instructions.append(nop)
        drain_inst.ins.sync_info = mybir.SyncInfo(
            on_wait=ow[-_MAX_WAITS:], on_update=list(si.on_update or [])
        )
        bb.instructions.append(drain_inst.ins)

    nc.all_engine_barrier()
    assert self.sems is not None
    popped = nc._tile_sem_poison_stack.pop()
    assert popped is self._sem_poison
    nc.clear_and_free_semaphores(list(self.sems.allocated().values()))
    nc.all_engine_barrier()


if not getattr(tile_mod.TileContext, "_ant_drain_patched", False):
    tile_mod.TileContext._drain_and_barrier = _patched_drain_and_barrier
    tile_mod.TileContext._ant_drain_patched = True


# ---------------------------------------------------------------------------
# Patch 2: NTFF profile hook shim (the image's antenv lacks axon_hooks);
# only needed when run with trace=True, harmless otherwise.
# ---------------------------------------------------------------------------
def _install_ntff_hook():
    if "antenv.axon_hooks" in sys.modules:
        return
    mod = types.ModuleType("antenv.axon_hooks")
    state = {"hook": None}
    mod.set_axon_ntff_profile_hook = lambda h: state.__setitem__("hook", h)
    mod.get_axon_ntff_profile_hook = lambda: state["hook"]
    sys.modules["antenv.axon_hooks"] = mod
    try:
        import antenv

        antenv.axon_hooks = mod
    except ImportError:
        pass
    try:
        from trn_agent_boot.trn_boot import _ntff_profile_via_ctypes

        mod.set_axon_ntff_profile_hook(
            _ntff_profile_via_ctypes("/opt/axon/libaxon_pjrt.so")
        )
    except Exception:
        pass


_install_ntff_hook()

# ---------------------------------------------------------------------------
# Problem constants (hardcoded per spec)
# ---------------------------------------------------------------------------
B, N, C, G, H, DK = 8, 128, 128, 2, 4, 32
BN_SCALE = float(1.0 / np.sqrt(1.0 + 1e-5))
NCHUNK = 32          # token chunks per psim (512 pair-tokens each)
SLABS = 4            # i-slabs per chunk
F32 = mybir.dt.float32
F32R = mybir.dt.float32r
F16 = mybir.dt.float16
ALU = mybir.AluOpType
ACT = mybir.ActivationFunctionType


def _psim_weight_names(s):
    return [f"w1_{s}", f"w2a_{s}", f"w2b_{s}", f"w3_{s}", f"b3_{s}"]


def _gen_weight_names(g):
    return [f"wq_{g}", f"wk_{g}", f"d1k0_{g}", f"d1k1_{g}", f"d2k0_{g}", f"d2k1_{g}"]


BLOB32 = [
    ("ident", N), ("omeye", N), ("eye_eps", N),
    ("b3_pre", 1), ("b3_g0", 1), ("b3_g1", 1),
    ("wq_0", N), ("wk_0", N), ("d1k0_0", 2 * C), ("d1k1_0", 2 * C),
    ("d2k0_0", C), ("d2k1_0", C),
    ("wq_1", N), ("wk_1", N), ("d1k0_1", 2 * C), ("d1k1_1", 2 * C),
    ("d2k0_1", C), ("d2k1_1", C),
]
BLOB16 = [
    ("w1_pre", 2 * C), ("w2a_pre", C), ("w2b_pre", C), ("w3_pre", 1),
    ("w1_g0", 2 * C), ("w2a_g0", C), ("w2b_g0", C), ("w3_g0", 1),
    ("w1_g1", 2 * C), ("w2a_g1", C), ("w2b_g1", C), ("w3_g1", 1),
]
CORE32 = [("vpT", N), ("vp_rm", C), ("ep0", N)]


def _offsets(blob):
    out, off = {}, 0
    for name, w in blob:
        out[name] = (off, w)
        off += w
    return out, off


OFF32, TOT32 = _offsets(BLOB32)
OFF16, TOT16 = _offsets(BLOB16)
OFFC, TOTC = _offsets(CORE32)


class _Pools:
    pass


_BLK = 16


def _tri_chunks():
    """Block-triangular chunk table: (i0, k, j0, w) — rows i0..i0+k,
    cols j0..j0+w, exploiting z's exact symmetry (z[i,j] == z[j,i])."""
    chunks = []
    for b in range(N // _BLK):
        j0 = _BLK * b
        w = N - j0
        kmax = max(1, min(_BLK, 512 // w))
        kmin = max(1, -(-256 // w))  # ceil(256/w)
        rem = _BLK
        i0 = j0
        while rem:
            k = min(kmax, rem)
            if 0 < rem - k < kmin:
                k = rem - kmin
            chunks.append((i0, k, j0, w))
            i0 += k
            rem -= k
    return chunks


_TRI = _tri_chunks()


def _emit_psim(nc, P, sb, cfg, vpT, ep_prev, nb, stage):
    """ep_new = PointSimilarity(vpT, ep_prev); nb = #smallest entries to zap."""
    from concourse.bass_types import AP as _AP

    zsb = sb.tile([N, N], F32, tag="zsb")
    zline = sb.tile([1, N * N], F32, tag="zline")
    # ep_nd/ep_sum depend only on ep_prev: compute early, overlaps the chunks
    ep_nd = sb.tile([N, N], F32, tag="ep_nd")
    ep_sum = sb.tile([N, 1], F32, tag="ep_sum")
    nc.vector.scalar_tensor_tensor(
        out=ep_nd[:], in0=ep_prev[:], scalar=1.0, in1=P.omeye[:],
        op0=ALU.mult, op1=ALU.mult, accum_out=ep_sum[:],
    )
    off = 0
    for (i0, k, j0, w) in _TRI:
        nt = k * w
        d = P.work.tile([C, 512], F32, tag="d")
        a = vpT[:]
        rep = _AP(a.tensor, a.offset + j0, [a.ap[0], [0, k], [1, w]])
        bcast = vpT[:, i0:i0 + k].to_broadcast([C, k, w])
        nc.vector.tensor_tensor(
            out=d[:, 0:nt].rearrange("p (s n) -> p s n", s=k),
            in0=rep, in1=bcast, op=ALU.subtract,
        )
        sim = P.work.tile([C, 512], F16, tag="sim")
        nc.gpsimd.tensor_tensor(out=sim[:, 0:nt], in0=d[:, 0:nt], in1=d[:, 0:nt],
                                op=ALU.mult)
        p1 = P.p1.tile([128, 1024], F32, tag="p1")
        nc.tensor.matmul(p1[:, 0:nt], cfg["w1r"][:, 0:128], sim[:, 0:nt],
                         start=True, stop=True)
        nc.tensor.matmul(p1[:, 512:512 + nt], cfg["w1r"][:, 128:256], sim[:, 0:nt],
                         start=True, stop=True)
        h1 = P.work.tile([128, 1024], F16, tag="h1")
        nc.scalar.activation(
            h1[:, 0:2 * nt].rearrange("p (a n) -> p a n", a=2),
            p1[:].rearrange("p (a n) -> p a n", a=2)[:, :, 0:nt],
            ACT.Lrelu, alpha=0.01,
        )
        p2 = P.p2.tile([128, 512], F32, tag="p2")
        nc.tensor.matmul(p2[:, 0:nt], cfg["w2ar"][:], h1[:, 0:nt],
                         start=True, stop=False)
        nc.tensor.matmul(p2[:, 0:nt], cfg["w2br"][:], h1[:, nt:2 * nt],
                         start=False, stop=True)
        h2 = P.work.tile([128, 512], F16, tag="h2")
        nc.scalar.activation(h2[:, 0:nt], p2[:, 0:nt], ACT.Lrelu, alpha=0.01)
        pz = P.pz.tile([1, 512], F32, tag="pz")
        nc.tensor.matmul(pz[:, 0:nt], cfg["w3r"][:], h2[:, 0:nt],
                         start=True, stop=True)
        nc.vector.tensor_copy(zline[:, off:off + nt], pz[:, 0:nt])
        # strip -> zsb block rows [i0:i0+k, j0:j0+w]
        nc.sync.dma_start(zsb[i0:i0 + k, j0:j0 + w], zline[:, off:off + nt])
        off += nt

    # mirror the lower triangle: zsb[i,j] = zsb[j,i] where j < 16*(i//16)
    pT = P.aux.tile([128, 512], F32, tag="pz")
    nc.tensor.transpose(pT[:, 0:128], zsb[:], P.ident[:])
    nc.vector.copy_predicated(zsb[:], P.masklow[:], pT[:, 0:128])

    e = sb.tile([N, N], F32, tag="esig")
    nc.scalar.activation(e[:], zsb[:], ACT.Sigmoid, bias=cfg["b3"][:], scale=1.0)
    s1 = sb.tile([N, 1], F32, tag="l1_s")
    if nb:
        nc.vector.tensor_tensor(out=e[:], in0=e[:], in1=ep_nd[:], op=ALU.mult)
    else:
        nc.vector.scalar_tensor_tensor(out=e[:], in0=e[:], scalar=1.0,
                                       in1=ep_nd[:], op0=ALU.bypass, op1=ALU.mult,
                                       accum_out=s1[:])

    if nb:
        # zap the nb smallest entries per row: run max8/match_replace on 2-e
        x = sb.tile([N, N], F32, tag="topk_x")
        nc.vector.tensor_scalar(out=x[:], in0=e[:], scalar1=-1.0, scalar2=2.0,
                                op0=ALU.mult, op1=ALU.add)
        mx = sb.tile([N, 8], F32, tag="topk_mx")
        done = 0
        while done < nb:
            k = min(8, nb - done)
            nc.vector.max(out=mx[:], in_=x[:])
            if k < 8:
                nc.vector.memset(mx[:, k:], 0.0)
            nc.vector.match_replace(out=x[:], in_to_replace=mx[:], in_values=x[:],
                                    imm_value=0.0)
            done += k
        m = sb.tile([N, N], F32, tag="topk_m")
        nc.vector.tensor_scalar(out=m[:], in0=x[:], scalar1=0.5, scalar2=None,
                                op0=ALU.is_gt)
        nc.vector.scalar_tensor_tensor(out=e[:], in0=e[:], scalar=1.0, in1=m[:],
                                       op0=ALU.bypass, op1=ALU.mult,
                                       accum_out=s1[:])

    rinv = sb.tile([N, 1], F32, tag="l1_rinv")
    nc.vector.reciprocal(rinv[:], s1[:])
    scl = sb.tile([N, 1], F32, tag="l1_scl")
    nc.vector.tensor_tensor(out=scl[:], in0=rinv[:], in1=ep_sum[:], op=ALU.mult)
    e2 = sb.tile([N, N], F32, tag="e2")
    rs = sb.tile([N, 1], F32, tag="rs")
    nc.vector.scalar_tensor_tensor(
        out=e2[:], in0=e[:], scalar=scl[:], in1=P.eye_eps[:],
        op0=ALU.mult, op1=ALU.add, accum_out=rs[:],
    )
    r2 = sb.tile([N, 1], F32, tag="r2")
    nc.vector.reciprocal(r2[:], rs[:])
    ep_new = sb.tile([N, N], F32, tag=f"ep_{stage}")
    nc.vector.tensor_scalar(out=ep_new[:], in0=e2[:], scalar1=r2[:], scalar2=None,
                            op0=ALU.mult)
    return ep_new


def _emit_mha(nc, P, sb, g, vpT, wq, wk):
    pqk = P.aux.tile([128, 512], F32, tag="pz")
    nc.tensor.matmul(pqk[:, 0:128], wq[:], vpT[:], start=True, stop=True)
    nc.tensor.matmul(pqk[:, 128:256], wk[:], vpT[:], start=True, stop=True)
    qkT = sb.tile([C, 2 * N], F32, tag="qkT")
    nc.vector.tensor_copy(qkT[:], pqk[:, 0:256])
    # per-head strips to partitions 0:32 (DMA partition reshape)
    qkh = sb.tile([DK, H, 2 * N], F32, tag="qkh")
    for h in range(H):
        nc.sync.dma_start(qkh[0:DK, h, :], qkT[DK * h:DK * (h + 1), :])
    attn = sb.tile([N, N], F32, tag=f"attn_{g}")
    inv_sqrt_dk = float(1.0 / np.sqrt(DK))
    # all 4 head scores into ONE psum bank, then the 4 Exps back-to-back
    # (minimizes ACT table swaps against the surrounding Lrelu stream)
    ps = P.aux.tile([128, 512], F32, tag="pz")
    for h in range(H):
        nc.tensor.matmul(ps[:, 128 * h:128 * (h + 1)],
                         qkh[0:DK, h, 0:N], qkh[0:DK, h, N:2 * N],
                         start=True, stop=True)
    mxs = sb.tile([N, H], F32, tag="mha_mx")
    nbias = sb.tile([N, H], F32, tag="mha_nb")
    nc.vector.tensor_reduce(
        mxs[:], ps[:].rearrange("p (h n) -> p h n", h=H),
        axis=mybir.AxisListType.X, op=ALU.max)
    nc.vector.tensor_scalar(out=nbias[:], in0=mxs[:], scalar1=-inv_sqrt_dk,
                            scalar2=None, op0=ALU.mult)
    ehs, ses = [], []
    for h in range(H):
        eh = sb.tile([N, N], F32, tag=f"mha_eh{h}")
        se = sb.tile([N, 1], F32, tag=f"mha_se{h}")
        nc.scalar.activation(eh[:], ps[:, 128 * h:128 * (h + 1)], ACT.Exp,
                             bias=nbias[:, h:h + 1], scale=inv_sqrt_dk,
                             accum_out=se[:])
        ehs.append(eh)
        ses.append(se)
    for h in range(H):
        rc = sb.tile([N, 1], F32, tag="mha_rc")
        nc.vector.reciprocal(rc[:], ses[h][:])
        nc.vector.tensor_scalar(out=rc[:], in0=rc[:], scalar1=1.0 / H,
                                scalar2=None, op0=ALU.mult)
        if h == 0:
            nc.vector.tensor_scalar(out=attn[:], in0=ehs[h][:], scalar1=rc[:],
                                    scalar2=None, op0=ALU.mult)
        else:
            nc.vector.scalar_tensor_tensor(out=attn[:], in0=ehs[h][:], scalar=rc[:],
                                           in1=attn[:], op0=ALU.mult, op1=ALU.add)
    # pre-kill the diagonal (d2p's edge needs (ep*attn)*(1-eye))
    nc.vector.tensor_tensor(out=attn[:], in0=attn[:], in1=P.omeye[:], op=ALU.mult)
    return attn


def _emit_d2p(nc, P, sb, g, ep, attn, vpT, vp_rm, cfg, last):
    edge = sb.tile([N, N], F32, tag="edge")
    s = sb.tile([N, 1], F32, tag="d2p_s")
    nc.vector.scalar_tensor_tensor(out=edge[:], in0=ep[:], scalar=1.0, in1=attn[:],
                                   op0=ALU.bypass, op1=ALU.mult, accum_out=s[:])
    rr = sb.tile([N, 1], F32, tag="d2p_rr")
    nc.vector.reciprocal(rr[:], s[:])
    nc.vector.tensor_scalar(out=edge[:], in0=edge[:], scalar1=rr[:], scalar2=None,
                            op0=ALU.mult)
    pT = P.aux.tile([128, 512], F32, tag="pz")
    nc.tensor.transpose(pT[:, 0:128], edge[:], P.ident[:])
    edgeT = sb.tile([N, N], F32, tag="edgeT")
    nc.vector.tensor_copy(edgeT[:], pT[:, 0:128])
    pa = P.aux.tile([128, 512], F32, tag="pz")
    nc.tensor.matmul(pa[:, 0:128], vp_rm[:], edgeT[:], start=True, stop=True)
    aggrT = sb.tile([C, N], F32, tag="aggrT")
    nc.vector.tensor_copy(aggrT[:], pa[:, 0:128])
    hs = []
    for oh in range(2):
        pm = P.aux.tile([128, 512], F32, tag="pz")
        nc.tensor.matmul(pm[:, 0:128], cfg[f"d1k0"][:, C * oh:C * (oh + 1)], vpT[:],
                         start=True, stop=False)
        nc.tensor.matmul(pm[:, 0:128], cfg[f"d1k1"][:, C * oh:C * (oh + 1)], aggrT[:],
                         start=False, stop=True)
        h_oh = sb.tile([C, N], F32, tag=f"d2ph{oh}")
        nc.scalar.activation(h_oh[:], pm[:, 0:128], ACT.Lrelu, alpha=0.01)
        hs.append(h_oh)
    pv = P.aux.tile([128, 512], F32, tag="pz")
    nc.tensor.matmul(pv[:, 0:128], cfg["d2k0"][:], hs[0][:], start=True, stop=False)
    nc.tensor.matmul(pv[:, 0:128], cfg["d2k1"][:], hs[1][:], start=False, stop=True)
    vpT_new = sb.tile([C, N], F32, tag=f"vpT_{g + 1}")
    nc.scalar.activation(vpT_new[:], pv[:, 0:128], ACT.Lrelu, alpha=0.01)
    vp_rm_new = None
    if not last:
        pt2 = P.aux.tile([128, 512], F32, tag="pz")
        nc.tensor.transpose(pt2[:, 0:128], vpT_new[:], P.ident[:])
        vp_rm_new = sb.tile([N, C], F32, tag=f"vprm_{g + 1}")
        nc.vector.tensor_copy(vp_rm_new[:], pt2[:, 0:128])
    return vpT_new, vp_rm_new


def build():
    """Build the single-core Bass graph (SPMD across 8 cores)."""
    nc = bass.Bass()
    b32_ext = nc.declare_dram_parameter("blob32", [128, TOT32], F32, isOutput=False)
    b16_ext = nc.declare_dram_parameter("blob16", [128, TOT16], F16, isOutput=False)
    core_ext = nc.declare_dram_parameter("core32", [128, TOTC], F32, isOutput=False)
    mask_ext = nc.declare_dram_parameter("masklow", [N, N], mybir.dt.uint8,
                                         isOutput=False)
    out_ext = nc.declare_dram_parameter("out", [N, N], F32, isOutput=True)

    with tile_mod.TileContext(nc) as tc:
        with tc.tile_pool(name="const", bufs=1) as const, \
             tc.tile_pool(name="state", bufs=1) as state, \
             tc.tile_pool(name="work", bufs=2) as work, \
             tc.tile_pool(name="p1", bufs=2, space="PSUM") as p1, \
             tc.tile_pool(name="p2", bufs=2, space="PSUM") as p2, \
             tc.tile_pool(name="pz", bufs=2, space="PSUM") as pz:

            P = _Pools()
            P.work, P.p1, P.p2, P.pz, P.aux = work, p1, p2, pz, pz

            # --- four bulk loads (small, psim-critical ones first) ---
            core = const.tile([128, TOTC], F32, tag="core")
            nc.sync.dma_start(core[:], core_ext[:])
            b16 = const.tile([128, TOT16], F16, tag="b16")
            nc.sync.dma_start(b16[:], b16_ext[:])
            P.masklow = const.tile([N, N], mybir.dt.uint8, tag="masklow")
            nc.sync.dma_start(P.masklow[:], mask_ext[:])
            b32 = const.tile([128, TOT32], F32, tag="b32")
            nc.sync.dma_start(b32[:], b32_ext[:])

            def s32(name):
                off, w = OFF32[name]
                return b32[:, off:off + w]

            def s16(name):
                off, w = OFF16[name]
                return b16[:, off:off + w]

            def sc(name):
                off, w = OFFC[name]
                return core[:, off:off + w]

            P.ident = s32("ident")
            P.omeye = s32("omeye")
            P.eye_eps = s32("eye_eps")

            vpT0, vprm0, ep0 = sc("vpT"), sc("vp_rm"), sc("ep0")

            psim_cfgs = {
                s: {"w1r": s16(f"w1_{s}"), "w2ar": s16(f"w2a_{s}"),
                    "w2br": s16(f"w2b_{s}"), "w3r": s16(f"w3_{s}"),
                    "b3": s32(f"b3_{s}")}
                for s in ("pre", "g0", "g1")
            }
            gen_cfgs = {
                g: {k: s32(f"{k}_{g}")
                    for k in ("wq", "wk", "d1k0", "d1k1", "d2k0", "d2k1")}
                for g in range(G)
            }

            # --- the pipeline (mha_g emitted after the psim it overlaps, so
            # the psim matmuls stay at the head of the in-order PE stream) ---
            ep = _emit_psim(nc, P, state, psim_cfgs["pre"], vpT0, ep0, 0, "pre")
            attn = _emit_mha(nc, P, state, 0, vpT0, gen_cfgs[0]["wq"],
                             gen_cfgs[0]["wk"])
            vpT, vp_rm = vpT0, vprm0
            for g in range(G):
                vpT, vp_rm = _emit_d2p(nc, P, state, g, ep, attn, vpT, vp_rm,
                                       gen_cfgs[g], last=(g == G - 1))
                if g + 1 < G:
                    attn = _emit_mha(nc, P, state, g + 1, vpT,
                                     gen_cfgs[g + 1]["wq"], gen_cfgs[g + 1]["wk"])
                kval = int(N * (1.0 - 0.1 * (g + 1)))
                ep = _emit_psim(nc, P, state, psim_cfgs[f"g{g}"], vpT, ep,
                                N - kval, f"g{g}")

            nc.sync.dma_start(out_ext[:], ep[:])

    return nc


def make_in_maps(vp, ep0, ps_pre_w1, ps_pre_w2, ps_pre_w3, ps_pre_b3,
                 ps_w1, ps_w2, ps_w3, ps_b3, d2p_w1, d2p_w2, mha_wq, mha_wk):
    f = np.float32
    a = lambda x: np.ascontiguousarray(np.asarray(x), dtype=f)
    eye = np.eye(N, dtype=f)
    ii = np.arange(N)
    masklow = np.ascontiguousarray(
        (ii[None, :] < (ii[:, None] // 16) * 16).astype(np.uint8))

    vals32 = {
        "ident": eye, "omeye": a(1.0 - eye), "eye_eps": a(eye + 1e-6),
    }
    vals16 = {}
    for s, w1, w2, w3, b3 in [
        ("pre", ps_pre_w1, ps_pre_w2, ps_pre_w3, ps_pre_b3),
        ("g0", ps_w1[0], ps_w2[0], ps_w3[0], ps_b3[0]),
        ("g1", ps_w1[1], ps_w2[1], ps_w3[1], ps_b3[1]),
    ]:
        w2s = a(np.asarray(w2) * BN_SCALE)
        vals16[f"w1_{s}"] = a(np.asarray(w1) * BN_SCALE)
        vals16[f"w2a_{s}"] = w2s[:C]
        vals16[f"w2b_{s}"] = w2s[C:]
        vals16[f"w3_{s}"] = a(w3)
        vals32[f"b3_{s}"] = a(np.broadcast_to(np.asarray(b3).reshape(1, 1), (N, 1)))
    for g in range(G):
        d1s = a(np.asarray(d2p_w1[g]) * BN_SCALE)
        d2s = a(np.asarray(d2p_w2[g]) * BN_SCALE)
        vals32[f"wq_{g}"] = a(mha_wq[g])
        vals32[f"wk_{g}"] = a(mha_wk[g])
        vals32[f"d1k0_{g}"] = d1s[:C]
        vals32[f"d1k1_{g}"] = d1s[C:]
        vals32[f"d2k0_{g}"] = d2s[:C]
        vals32[f"d2k1_{g}"] = d2s[C:]

    blob32 = np.zeros((128, TOT32), dtype=f)
    for name, w in BLOB32:
        off = OFF32[name][0]
        blob32[:, off:off + w] = vals32[name]
    blob16 = np.zeros((128, TOT16), dtype=np.float16)
    for name, w in BLOB16:
        off = OFF16[name][0]
        blob16[:, off:off + w] = vals16[name].astype(np.float16)

    shared = {"blob32": blob32, "blob16": blob16, "masklow": masklow}
    vp = a(vp)
    ep0 = a(ep0)
    in_maps = []
    for i in range(B):
        cb = np.zeros((128, TOTC), dtype=f)
        cb[:, OFFC["vpT"][0]:OFFC["vpT"][0] + N] = vp[i].T
        cb[:, OFFC["vp_rm"][0]:OFFC["vp_rm"][0] + C] = vp[i]
        cb[:, OFFC["ep0"][0]:OFFC["ep0"][0] + N] = ep0[i]
        m = dict(shared)
        m["core32"] = cb
        in_maps.append(m)
    return in_maps


_CACHED_NC = None


def _get_nc():
    global _CACHED_NC
    if _CACHED_NC is None:
        _CACHED_NC = build()
    return _CACHED_NC


def run(in_maps, trace=False):
    from concourse.bass_utils import run_bass_kernel_spmd

    nc = _get_nc()
    return run_bass_kernel_spmd(nc, in_maps, list(range(B)), trace=trace)


def kernel(**inputs) -> np.ndarray:
    in_maps = make_in_maps(**inputs)
    r = run(in_maps, trace=False)
    return np.stack([r.results[i]["out"] for i in range(B)]).astype(np.float32)
